# revision 45
# baseline (speedup 1.0000x reference)
"""Channel Attention Module (CAM) TRN2 Bass kernel.

Reference (per batch b of x[B, H, W, C], B=16, H=W=64, C=256):
    a    = x[b].reshape(HW, C)
    G    = a.T @ a                      # [C, C] gram
    attn = softmax(G, axis=-1)
    out  = gamma * (a @ attn) + x[b]

Sharding: data parallel over batch, 16 batches across 8 NeuronCores ->
2 batches per core, no cross-core communication.  kernel() takes the
full inputs, shards, runs SPMD on cores 0-7, and reassembles.

Per-core schedule (matmuls in bf16, accumulation/softmax in fp32):
  input   x rows are laid out as n = p*32 + j (partition p, free j), so
          every DMA line is one contiguous 4 KB block per partition.
          The gram, softmax and second matmul are invariant to this
          permutation of n; the output DMA mirrors it.
  stage A per 128-row chunk: cast to bf16 (DVE), two gram matmuls
          accumulating G in PSUM, two transpose matmuls against the
          identity (the same stationary operand as the gram matmuls)
          producing aT, whose PSUM->SBUF copy (ACT 2/3, DVE 1/3) also
          casts to bf16.
  stage B row softmax of G: reduce_max(negate) -> Exp with per-partition
          bias and fused row-sum -> reciprocal -> scale; 1/rowsum and
          gamma are folded into attn so the epilogue is a plain add.
  stage C per chunk pair: psum_O = aT.T @ attn (4 matmuls, one PSUM
          bank), epilogue out = psum_O + x on DVE, one output DMA per
          4 chunks.
  Phase order A0, A1, C0, C1: both softmaxes hide under PE work and the
  output stream starts right as the input stream drains.
"""

import numpy as np

P = 128
C = 256
HW = 4096
NCH = HW // P          # 32 row-chunks per batch
BPC = 2                # batches per core
GRP = 4                # chunks per output DMA group
N_CORES = 8


def _fix_bir_json(raw: bytes) -> bytes:
    """Post-process the serialized BIR before it reaches the compiler.

    (1) Pending PSUM-slot WAR guards materialize as wait-carrying Drain
    instructions on the PE sequencer; a Drain empties the PE pipe, which
    serializes dispatch every chunk and keeps the HAM clock gate at
    1.2 GHz.  A dispatch-level wait (NoOp+wait) is sufficient for a WAR
    hazard -- consumer semaphores increment at completion and each
    engine executes in order -- so rewrite wait-only non-reset Drains in
    the main body as NoOps.
    (2) walrus's CoreV3 codegen rejects >1 semaphore wait on one
    instruction; hoist extra waits onto preceding NoOps.
    """
    import orjson

    m = orjson.loads(raw)
    ctr = [0]

    def mk_nop(engine, waits, debug):
        ctr[0] += 1
        nop = {
            "engine": engine,
            "ins": [],
            "name": f"I-waitfix-{ctr[0]}",
            "opcode": "NoOp",
            "outs": [],
            "sync_info": {"on_update": [], "on_wait": waits},
        }
        if debug is not None:
            nop["debug"] = debug
        return nop

    for fn in m["functions"]:
        for b in fn["blocks"]:
            is_end = b["name"].endswith("_end")
            out = []
            for inst in b["instructions"]:
                si = inst.get("sync_info") or {}
                waits = si.get("on_wait") or []
                ups = si.get("on_update") or []
                if (
                    inst.get("opcode") == "Drain"
                    and not is_end
                    and waits
                    and not ups
                    and not inst.get("is_reset_sema")
                ):
                    inst = mk_nop(inst["engine"], waits, inst.get("debug"))
                    si = inst["sync_info"]
                if len(waits) > 1:
                    for w in waits[:-1]:
                        out.append(mk_nop(inst["engine"], [w], inst.get("debug")))
                    si = dict(si)
                    si["on_wait"] = [waits[-1]]
                    inst["sync_info"] = si
                out.append(inst)
            b["instructions"] = out
    return orjson.dumps(m)


def _build():
    import concourse.bass as bass
    import concourse.tile as tile
    from concourse import mybir
    from concourse.masks import make_identity

    f32 = mybir.dt.float32
    bf16 = mybir.dt.bfloat16
    nc = bass.Bass("TRN2", target_bir_lowering=False, debug=False)

    x_ext = nc.declare_dram_parameter("x", [BPC, HW, C], f32, isOutput=False)
    g_ext = nc.declare_dram_parameter("gamma", [1], f32, isOutput=False)
    out_ext = nc.declare_dram_parameter("out", [BPC, HW, C], f32, isOutput=True)

    with tile.TileContext(nc) as tc:
        with (
            tc.tile_pool(name="const", bufs=1) as const_pool,
            tc.tile_pool(name="a", bufs=2) as a_pool,
            tc.tile_pool(name="abf", bufs=2) as abf_pool,
            tc.tile_pool(name="at", bufs=2) as at_pool,
            tc.tile_pool(name="attn", bufs=2) as attn_pool,
            tc.tile_pool(name="small", bufs=2) as small_pool,
            tc.tile_pool(name="outs", bufs=4) as out_pool,
            tc.tile_pool(name="psG", bufs=2, space="PSUM") as psG_pool,
            tc.tile_pool(name="psT", bufs=3, space="PSUM") as psT_pool,
            tc.tile_pool(name="psO", bufs=3, space="PSUM") as psO_pool,
        ):
            ident = const_pool.tile([P, P], bf16)
            make_identity(nc, ident[:])

            # gamma -> all 128 partitions (step-0 DMA broadcast)
            gamma_bc = const_pool.tile([P, 1], f32)
            nc.sync.dma_start(gamma_bc[:], g_ext[None, :].to_broadcast((P, 1)))

            # All input DMAs are issued up front on SP, before any
            # output-side waits enter the SP stream.
            DGRP = 4  # j-chunks per input DMA (4 KB per partition)
            a_sbs, a_bfs, at_sbs = [], [], []
            for b in range(BPC):
                xr = x_ext[b].rearrange("(p j) f -> p j f", p=P)
                a_sb = a_pool.tile([P, NCH, C], f32, tag="a_sb")
                a_sbs.append(a_sb)
                a_bfs.append(
                    abf_pool.tile([P, NCH, C], bf16, name="a_bf", tag="a_bf")
                )
                at_sbs.append(
                    at_pool.tile([P, 2, HW], bf16, name="at_sb", tag="at_sb")
                )
                for g in range(NCH // DGRP):
                    nc.sync.dma_start(
                        a_sb[:, bass.ts(g, DGRP), :], xr[:, bass.ts(g, DGRP), :]
                    )

            attns = []

            def emit_A_chunk(b, c, psum_G):
                a_sb, a_bf, at_sb = a_sbs[b], a_bfs[b], at_sbs[b]
                nc.vector.tensor_copy(a_bf[:, c, :], a_sb[:, c, :])
                psum_T = psT_pool.tile([P, C], f32, name="psum_T")
                for ic in range(2):
                    nc.tensor.matmul(
                        psum_G[:, bass.ts(ic, C)],
                        a_bf[:, c, bass.ts(ic, P)],
                        a_bf[:, c, :],
                        start=(c == 0),
                        stop=(c == NCH - 1),
                        skip_group_check=True,
                    )
                    nc.tensor.matmul(
                        psum_T[:, bass.ts(ic, P)],
                        a_bf[:, c, bass.ts(ic, P)],
                        ident[:],
                        start=True,
                        stop=True,
                        skip_group_check=True,
                    )
                # psum_T[i_loc, ic*128+n_loc] -> at_sb[i_loc, ic, c*128+n_loc]
                src_ap = psum_T[:].rearrange("p (ic n) -> p ic n", ic=2)
                dst = at_sb[:, :, bass.ts(c, P)]
                if c % 3 == 2:
                    nc.vector.tensor_copy(dst, src_ap)
                else:
                    nc.scalar.copy(dst, src_ap)

            def emit_softmax(b, psum_G):
                negmax = small_pool.tile([P, 2], f32, name="negmax")
                ssum = small_pool.tile([P, 2], f32, name="ssum")
                rg = small_pool.tile([P, 2], f32, name="rg")
                attn = attn_pool.tile([P, 2, C], bf16, name="attn")
                for ic in range(2):
                    nc.vector.reduce_max(
                        negmax[:, ic:ic + 1],
                        psum_G[:, bass.ts(ic, C)],
                        axis=mybir.AxisListType.X,
                        negate=True,
                    )
                E = attn_pool.tile([P, 2, C], f32, name="E")
                for ic in range(2):
                    nc.scalar.activation(
                        E[:, ic, :],
                        psum_G[:, bass.ts(ic, C)],
                        mybir.ActivationFunctionType.Exp,
                        bias=negmax[:, ic:ic + 1],
                        scale=1.0,
                        accum_out=ssum[:, ic:ic + 1],
                    )
                recip = small_pool.tile([P, 2], f32, name="recip")
                nc.vector.reciprocal(recip[:], ssum[:])
                nc.vector.tensor_scalar_mul(rg[:], recip[:], gamma_bc[:, 0:1])
                for ic in range(2):
                    nc.vector.tensor_scalar_mul(
                        attn[:, ic, :], E[:, ic, :], rg[:, ic:ic + 1]
                    )
                return attn

            out_state = {}

            def emit_C_pair(b, pr):
                # pair pr covers chunks 2*pr, 2*pr+1; GRP//2 pairs per
                # output DMA group
                a_sb, at_sb, attn = a_sbs[b], at_sbs[b], attns[b]
                outr = out_ext[b].rearrange("(p j) f -> p j f", p=P)
                if pr % (GRP // 2) == 0:
                    out_state[b] = out_pool.tile([P, GRP, C], f32, name="out_sb")
                out_sb = out_state[b]
                c = pr * 2
                cp = pr % (GRP // 2)
                psum_O = psO_pool.tile([P, 2 * C], f32, name="psum_O")
                for cc in range(2):
                    for ic in range(2):
                        nc.tensor.matmul(
                            psum_O[:, bass.ts(cc, C)],
                            at_sb[:, ic, bass.ts(c + cc, P)],
                            attn[:, ic, :],
                            start=(ic == 0),
                            stop=(ic == 1),
                        )
                nc.vector.tensor_tensor(
                    out_sb[:, cp * 2:cp * 2 + 2, :],
                    psum_O[:].rearrange("p (cc f) -> p cc f", cc=2),
                    a_sb[:, c:c + 2, :],
                    mybir.AluOpType.add,
                )
                if pr % (GRP // 2) == (GRP // 2) - 1:
                    g = pr // (GRP // 2)
                    nc.sync.dma_start(outr[:, bass.ts(g, GRP), :], out_sb[:])

            psum_G0 = psG_pool.tile([P, 2 * C], f32, name="psum_G")
            for c in range(NCH):
                emit_A_chunk(0, c, psum_G0)
            attns.append(emit_softmax(0, psum_G0))
            # first output group of batch 0 jumps the queue so the output
            # stream starts as the input stream drains
            for pr in range(2):
                emit_C_pair(0, pr)
            psum_G1 = psG_pool.tile([P, 2 * C], f32, name="psum_G")
            for c in range(NCH):
                emit_A_chunk(1, c, psum_G1)
            attns.append(emit_softmax(1, psum_G1))
            for pr in range(2, NCH // 2):
                emit_C_pair(0, pr)
            for pr in range(NCH // 2):
                emit_C_pair(1, pr)

    return nc


_NC = None


def _get_nc():
    global _NC
    if _NC is None:
        nc = _build()
        # Serialize once, post-process the JSON, and pin the result: the
        # run path fetches the BIR via nc.to_json_bytes(), and pending
        # sync deps materialize nondeterministically at serialization
        # time -- fixing the serialized form is the deterministic hook.
        fixed = _fix_bir_json(type(nc).to_json_bytes(nc))
        nc.to_json_bytes = lambda: fixed
        _NC = nc
    return _NC


def kernel(x: np.ndarray, gamma: np.ndarray) -> np.ndarray:
    from concourse.bass_utils import run_bass_kernel_spmd

    B, H, W, Cc = x.shape
    assert (B, H, W, Cc) == (16, 64, 64, 256)
    nc = _get_nc()
    xs = np.ascontiguousarray(
        x.reshape(N_CORES, BPC, HW, C).astype(np.float32, copy=False)
    )
    gamma = np.ascontiguousarray(gamma.astype(np.float32, copy=False))
    in_maps = [{"x": xs[i], "gamma": gamma} for i in range(N_CORES)]
    res = run_bass_kernel_spmd(nc, in_maps, core_ids=list(range(N_CORES)))
    out = np.stack([res.results[i]["out"] for i in range(N_CORES)])
    return out.reshape(B, H, W, Cc)


# revision 46
# speedup vs baseline: 1.0240x; 1.0240x over previous
"""Channel Attention Module (CAM) TRN2 Bass kernel.

Reference (per batch b of x[B, H, W, C], B=16, H=W=64, C=256):
    a    = x[b].reshape(HW, C)
    G    = a.T @ a                      # [C, C] gram
    attn = softmax(G, axis=-1)
    out  = gamma * (a @ attn) + x[b]

Sharding: data parallel over batch, 16 batches across 8 NeuronCores ->
2 batches per core, no cross-core communication.  kernel() takes the
full inputs, shards, runs SPMD on cores 0-7, and reassembles.

Per-core schedule (matmuls in bf16, accumulation/softmax in fp32):
  input   x rows are laid out as n = p*32 + j (partition p, free j), so
          every DMA line is one contiguous 4 KB block per partition.
          The gram, softmax and second matmul are invariant to this
          permutation of n; the output DMA mirrors it.
  stage A per 128-row chunk: cast to bf16 (DVE), two gram matmuls
          accumulating G in PSUM, two transpose matmuls against the
          identity (the same stationary operand as the gram matmuls)
          producing aT, whose PSUM->SBUF copy (ACT 2/3, DVE 1/3) also
          casts to bf16.
  stage B row softmax of G: reduce_max(negate) -> Exp with per-partition
          bias and fused row-sum -> reciprocal -> scale; 1/rowsum and
          gamma are folded into attn so the epilogue is a plain add.
  stage C per chunk pair: psum_O = aT.T @ attn (4 matmuls, one PSUM
          bank), epilogue out = psum_O + x on DVE, one output DMA per
          4 chunks.
  Phase order A0, A1, C0, C1: both softmaxes hide under PE work and the
  output stream starts right as the input stream drains.
"""

import numpy as np

P = 128
C = 256
HW = 4096
NCH = HW // P          # 32 row-chunks per batch
BPC = 2                # batches per core
GRP = 4                # chunks per output DMA group
N_CORES = 8


def _fix_bir_json(raw: bytes) -> bytes:
    """Post-process the serialized BIR before it reaches the compiler.

    (1) Pending PSUM-slot WAR guards materialize as wait-carrying Drain
    instructions on the PE sequencer; a Drain empties the PE pipe, which
    serializes dispatch every chunk and keeps the HAM clock gate at
    1.2 GHz.  A dispatch-level wait (NoOp+wait) is sufficient for a WAR
    hazard -- consumer semaphores increment at completion and each
    engine executes in order -- so rewrite wait-only non-reset Drains in
    the main body as NoOps.
    (2) walrus's CoreV3 codegen rejects >1 semaphore wait on one
    instruction; hoist extra waits onto preceding NoOps.
    """
    import orjson

    m = orjson.loads(raw)
    ctr = [0]

    def mk_nop(engine, waits, debug):
        ctr[0] += 1
        nop = {
            "engine": engine,
            "ins": [],
            "name": f"I-waitfix-{ctr[0]}",
            "opcode": "NoOp",
            "outs": [],
            "sync_info": {"on_update": [], "on_wait": waits},
        }
        if debug is not None:
            nop["debug"] = debug
        return nop

    for fn in m["functions"]:
        for b in fn["blocks"]:
            is_end = b["name"].endswith("_end")
            out = []
            for inst in b["instructions"]:
                si = inst.get("sync_info") or {}
                waits = si.get("on_wait") or []
                ups = si.get("on_update") or []
                if (
                    inst.get("opcode") == "Drain"
                    and not is_end
                    and waits
                    and not ups
                    and not inst.get("is_reset_sema")
                ):
                    inst = mk_nop(inst["engine"], waits, inst.get("debug"))
                    si = inst["sync_info"]
                if len(waits) > 1:
                    for w in waits[:-1]:
                        out.append(mk_nop(inst["engine"], [w], inst.get("debug")))
                    si = dict(si)
                    si["on_wait"] = [waits[-1]]
                    inst["sync_info"] = si
                out.append(inst)
            b["instructions"] = out
    return orjson.dumps(m)


def _build():
    import concourse.bass as bass
    import concourse.tile as tile
    from concourse import mybir
    from concourse.masks import make_identity

    f32 = mybir.dt.float32
    bf16 = mybir.dt.bfloat16
    nc = bass.Bass("TRN2", target_bir_lowering=False, debug=False)

    x_ext = nc.declare_dram_parameter("x", [BPC, HW, C], f32, isOutput=False)
    g_ext = nc.declare_dram_parameter("gamma", [1], f32, isOutput=False)
    out_ext = nc.declare_dram_parameter("out", [BPC, HW, C], f32, isOutput=True)

    with tile.TileContext(nc) as tc:
        with (
            tc.tile_pool(name="const", bufs=1) as const_pool,
            tc.tile_pool(name="a", bufs=2) as a_pool,
            tc.tile_pool(name="abf", bufs=2) as abf_pool,
            tc.tile_pool(name="at", bufs=2) as at_pool,
            tc.tile_pool(name="attn", bufs=2) as attn_pool,
            tc.tile_pool(name="small", bufs=2) as small_pool,
            tc.tile_pool(name="outs", bufs=4) as out_pool,
            tc.tile_pool(name="psG", bufs=2, space="PSUM") as psG_pool,
            tc.tile_pool(name="psT", bufs=3, space="PSUM") as psT_pool,
            tc.tile_pool(name="psO", bufs=3, space="PSUM") as psO_pool,
        ):
            ident = const_pool.tile([P, P], bf16)
            make_identity(nc, ident[:])

            # gamma -> all 128 partitions (step-0 DMA broadcast)
            gamma_bc = const_pool.tile([P, 1], f32)
            nc.sync.dma_start(gamma_bc[:], g_ext[None, :].to_broadcast((P, 1)))

            # All input DMAs are issued up front on SP, before any
            # output-side waits enter the SP stream.
            DGRP = 4  # j-chunks per input DMA (4 KB per partition)
            a_sbs, a_bfs, at_sbs = [], [], []
            for b in range(BPC):
                xr = x_ext[b].rearrange("(p j) f -> p j f", p=P)
                a_sb = a_pool.tile([P, NCH, C], f32, tag="a_sb")
                a_sbs.append(a_sb)
                a_bfs.append(
                    abf_pool.tile([P, NCH, C], bf16, name="a_bf", tag="a_bf")
                )
                at_sbs.append(
                    at_pool.tile([P, 2, HW], bf16, name="at_sb", tag="at_sb")
                )
                for g in range(NCH // DGRP):
                    nc.sync.dma_start(
                        a_sb[:, bass.ts(g, DGRP), :], xr[:, bass.ts(g, DGRP), :]
                    )

            attns = []

            def emit_A_chunk(b, c, psum_G):
                a_sb, a_bf, at_sb = a_sbs[b], a_bfs[b], at_sbs[b]
                nc.vector.tensor_copy(a_bf[:, c, :], a_sb[:, c, :])
                psum_T = psT_pool.tile([P, C], f32, name="psum_T")
                for ic in range(2):
                    nc.tensor.matmul(
                        psum_G[:, bass.ts(ic, C)],
                        a_bf[:, c, bass.ts(ic, P)],
                        a_bf[:, c, :],
                        start=(c == 0),
                        stop=(c == NCH - 1),
                        skip_group_check=True,
                    )
                    nc.tensor.matmul(
                        psum_T[:, bass.ts(ic, P)],
                        a_bf[:, c, bass.ts(ic, P)],
                        ident[:],
                        start=True,
                        stop=True,
                        skip_group_check=True,
                    )
                # psum_T[i_loc, ic*128+n_loc] -> at_sb[i_loc, ic, c*128+n_loc]
                src_ap = psum_T[:].rearrange("p (ic n) -> p ic n", ic=2)
                dst = at_sb[:, :, bass.ts(c, P)]
                if c % 3 == 2:
                    nc.vector.tensor_copy(dst, src_ap)
                else:
                    nc.scalar.copy(dst, src_ap)

            def emit_softmax(b, psum_G):
                negmax = small_pool.tile([P, 2], f32, name="negmax")
                ssum = small_pool.tile([P, 2], f32, name="ssum")
                rg = small_pool.tile([P, 2], f32, name="rg")
                attn = attn_pool.tile([P, 2, C], bf16, name="attn")
                for ic in range(2):
                    nc.vector.reduce_max(
                        negmax[:, ic:ic + 1],
                        psum_G[:, bass.ts(ic, C)],
                        axis=mybir.AxisListType.X,
                        negate=True,
                    )
                E = attn_pool.tile([P, 2, C], f32, name="E")
                for ic in range(2):
                    nc.scalar.activation(
                        E[:, ic, :],
                        psum_G[:, bass.ts(ic, C)],
                        mybir.ActivationFunctionType.Exp,
                        bias=negmax[:, ic:ic + 1],
                        scale=1.0,
                        accum_out=ssum[:, ic:ic + 1],
                    )
                recip = small_pool.tile([P, 2], f32, name="recip")
                nc.vector.reciprocal(recip[:], ssum[:])
                nc.vector.tensor_scalar_mul(rg[:], recip[:], gamma_bc[:, 0:1])
                for ic in range(2):
                    nc.vector.tensor_scalar_mul(
                        attn[:, ic, :], E[:, ic, :], rg[:, ic:ic + 1]
                    )
                return attn

            out_state = {}

            def emit_C_pair(b, pr):
                # pair pr covers chunks 2*pr, 2*pr+1; GRP//2 pairs per
                # output DMA group
                a_sb, at_sb, attn = a_sbs[b], at_sbs[b], attns[b]
                outr = out_ext[b].rearrange("(p j) f -> p j f", p=P)
                if pr % (GRP // 2) == 0:
                    out_state[b] = out_pool.tile([P, GRP, C], f32, name="out_sb")
                out_sb = out_state[b]
                c = pr * 2
                cp = pr % (GRP // 2)
                psum_O = psO_pool.tile([P, 2 * C], f32, name="psum_O")
                for cc in range(2):
                    for ic in range(2):
                        nc.tensor.matmul(
                            psum_O[:, bass.ts(cc, C)],
                            at_sb[:, ic, bass.ts(c + cc, P)],
                            attn[:, ic, :],
                            start=(ic == 0),
                            stop=(ic == 1),
                        )
                nc.vector.tensor_tensor(
                    out_sb[:, cp * 2:cp * 2 + 2, :],
                    psum_O[:].rearrange("p (cc f) -> p cc f", cc=2),
                    a_sb[:, c:c + 2, :],
                    mybir.AluOpType.add,
                )
                if pr % (GRP // 2) == (GRP // 2) - 1:
                    g = pr // (GRP // 2)
                    nc.sync.dma_start(outr[:, bass.ts(g, GRP), :], out_sb[:])

            psum_G0 = psG_pool.tile([P, 2 * C], f32, name="psum_G")
            for c in range(NCH):
                emit_A_chunk(0, c, psum_G0)
            attns.append(emit_softmax(0, psum_G0))
            psum_G1 = psG_pool.tile([P, 2 * C], f32, name="psum_G")
            for c in range(NCH):
                emit_A_chunk(1, c, psum_G1)
            attns.append(emit_softmax(1, psum_G1))
            for pr in range(NCH // 2):
                emit_C_pair(0, pr)
            for pr in range(NCH // 2):
                emit_C_pair(1, pr)

    return nc


_NC = None


def _get_nc():
    global _NC
    if _NC is None:
        nc = _build()
        # Serialize once, post-process the JSON, and pin the result: the
        # run path fetches the BIR via nc.to_json_bytes(), and pending
        # sync deps materialize nondeterministically at serialization
        # time -- fixing the serialized form is the deterministic hook.
        fixed = _fix_bir_json(type(nc).to_json_bytes(nc))
        nc.to_json_bytes = lambda: fixed
        _NC = nc
    return _NC


def kernel(x: np.ndarray, gamma: np.ndarray) -> np.ndarray:
    from concourse.bass_utils import run_bass_kernel_spmd

    B, H, W, Cc = x.shape
    assert (B, H, W, Cc) == (16, 64, 64, 256)
    nc = _get_nc()
    xs = np.ascontiguousarray(
        x.reshape(N_CORES, BPC, HW, C).astype(np.float32, copy=False)
    )
    gamma = np.ascontiguousarray(gamma.astype(np.float32, copy=False))
    in_maps = [{"x": xs[i], "gamma": gamma} for i in range(N_CORES)]
    res = run_bass_kernel_spmd(nc, in_maps, core_ids=list(range(N_CORES)))
    out = np.stack([res.results[i]["out"] for i in range(N_CORES)])
    return out.reshape(B, H, W, Cc)
